# revision 1
# baseline (speedup 1.0000x reference)
"""Banded (sliding-window) multi-head attention for TRN2, 8 NeuronCores.

Problem: nn_BaseAttention (B=2, T=4096, C=512, H=8, hd=64, WIN=128).
  qkv = x @ W_qkv ; banded softmax(q k^T / sqrt(hd), |i-j|<=WIN) @ v ; @ W_out + b_out

Sharding: 8 cores = 2 batches x 4 T-chunks of 1024 queries. Each core gets its
x rows plus a 128-row halo on each side (zero-padded at sequence edges) and
full replicated weights; it produces its own [1024, 512] output slice, so the
host-side gather is pure concatenation (no cross-core reduction).

Device pipeline per core (all layouts chosen to avoid transposing activations):
  xT   = x^T arrives pre-transposed from host        [C, 1280]
  q^T/k^T = W-stationary matmuls                     [128 featpair, rows]
  v    = natural matmuls packed as [V_e | 1 | V_o | 1] stationary tiles
  S^T  = k^T-stationary matmuls, 256-query windows -> 2 packed PSUM banks
  exp on ACT (table pinned to natural_log_exp_and_others) -> es strips
  es *= band-mask (host-computed, includes sequence-edge kills)
  O^T  = V-stationary matmuls -> one [O|sums] PSUM bank for both heads
  rs = exp(-ln(sums)) on ACT ; oa = O * rs on DVE
  Y = oa-stationary @ W_out + rank-1 bias matmul ; f16 out.

The 256-query windows keep every PSUM stage double-buffered inside the
8-bank budget, so the S->exp->AV->normalize chain pipelines across slots and
the tensor queue never drains (keeps the PE HAM clock gate at 8/8).
"""

import numpy as np

import concourse.bass as bass
from concourse import bacc
import concourse.mybir as mybir
import concourse.tile as tile
from concourse.bass_utils import run_bass_kernel_spmd

# ----- problem constants (hardcoded per contest contract) -----
B, T, C = 2, 4096, 512
H, HD, WIN = 8, 64, 128
NCORES = 8
CHUNK = 1024                # queries per core
ROWS = CHUNK + 2 * WIN      # 1280 rows incl. halo
QW = 256                    # query-window width
NQC = CHUNK // QW           # 4 windows
NKT = 4                     # key tiles per window
SCALE = HD ** -0.5

F32 = mybir.dt.float32
F16 = mybir.dt.float16
EXP = mybir.ActivationFunctionType.Exp

# per-key-tile true geometry: query-col range [tc0, tc1) within the window
_KT_TC = [(0, 128), (0, 256), (0, 256), (128, 256)]
# psum packing: (bank, local col offset); bank0 = kt0|kt1, bank1 = kt2|kt3
_KT_BANK = [(0, 0), (0, 128), (1, 0), (1, 256)]
# es strip offsets (matches the packed bank order, 384 per bank)
_KT_OFF = [0, 128, 384, 640]
ES_W = 768

DEBUG_DUMPS = False


def build_attention_body(tc, y, xh, wqkv, wout, bout, masks, dbg=None):
    """Emit the per-core kernel. All APs are DRAM tensors.

    y     [1024, 512] f16 out   xh    [512, 1280] f16 in (halo'd x, pre-T)
    wqkv  [512, 1536] f16 in (q-block pre-scaled by hd^-0.5 on host)
    wout  [512, 512]  f16 in    bout  [1, 512] f16 in
    masks [128, 4*768] f16 in: per-window packed band masks (0/1),
          sequence-edge kills baked in by the host.
    """
    nc = tc.nc
    from contextlib import ExitStack

    with ExitStack() as ctx:
        sb = ctx.enter_context(tc.tile_pool(name="sb", bufs=1))
        pp = ctx.enter_context(tc.tile_pool(name="pp", bufs=1, space="PSUM"))

        # Pin the ACT table to natural_log_exp_and_others (id 6): it serves
        # exp, ln AND copy, so the table-load pass sees every activation
        # already servable and the exp<->ln ping-pong disappears.
        nc.scalar.add_instruction(mybir.InstLoadActFuncSet(
            name=nc.get_next_instruction_name(), ins=[], outs=[],
            act_func_set_id=6))

        # ---- constants / persistent tiles ----
        ones_f = sb.tile([128, 128], F32, tag="ones_f", name="ones_f")
        nc.gpsimd.memset(ones_f[:], 1.0)
        ones_b = sb.tile([1, 128], F16, tag="ones_b", name="ones_b")
        nc.vector.tensor_copy(ones_b[:], ones_f[0:1, :])
        # zero row: stationary/moving for the cheap full-bank PSUM openers
        zline = sb.tile([1, C], F16, tag="zline", name="zline")
        nc.gpsimd.memset(zline[:], 0.0)

        # split the x / W_qkv loads so the first projections start early;
        # v-block columns before the late-k columns
        xT = [sb.tile([128, ROWS], F16, tag=f"xT{i}", name=f"xT{i}") for i in range(4)]
        wq_sb = [sb.tile([128, 3 * C], F16, tag=f"wq{i}", name=f"wq{i}")
                 for i in range(4)]
        for ct in range(4):
            eng = nc.sync if ct % 2 == 0 else nc.scalar
            eng.dma_start(xT[ct][:, 0:640], xh[128 * ct:128 * (ct + 1), 0:640])
            eng.dma_start(wq_sb[ct][:, 0:768],
                          wqkv[128 * ct:128 * (ct + 1), 0:768])
        for ct in range(4):
            eng = nc.sync if ct % 2 == 0 else nc.scalar
            eng.dma_start(xT[ct][:, 640:ROWS],
                          xh[128 * ct:128 * (ct + 1), 640:ROWS])
            eng.dma_start(wq_sb[ct][:, 1024:3 * C],
                          wqkv[128 * ct:128 * (ct + 1), 1024:3 * C])
        for ct in range(4):
            eng = nc.sync if ct % 2 == 0 else nc.scalar
            eng.dma_start(wq_sb[ct][:, 768:1024],
                          wqkv[128 * ct:128 * (ct + 1), 768:1024])
        wo_sb = []
        for i in range(4):
            w_i = sb.tile([128, C], F16, tag=f"wo{i}", name=f"wo{i}")
            nc.gpsimd.dma_start(w_i[:], wout[128 * i:128 * (i + 1), :])
            wo_sb.append(w_i)
        bo = sb.tile([1, C], F16, tag="bo", name="bo")
        nc.gpsimd.dma_start(bo[:], bout[:])
        msk = sb.tile([128, NQC * ES_W], F16, tag="msk", name="msk")
        nc.gpsimd.dma_start(msk[:], masks[:])

        qT = [sb.tile([128, CHUNK], F16, tag=f"qT{i}", name=f"qT{i}") for i in range(4)]
        kT = [sb.tile([128, ROWS], F16, tag=f"kT{i}", name=f"kT{i}") for i in range(4)]
        # fused V/ones stationary tiles: per key-row-tile rt, 4 pair blocks of
        # 256 cols laid out [V_even | 1 | V_odd | 1] (64 each). j=0 uses cols
        # [0:128) = [V_e | 1]; j=1 uses [128:256) = [V_o | 1] — both head
        # sums land in output partitions 64:128 so ln/exp normalize fuses.
        vpx = [sb.tile([128, 1024], F16, tag=f"vpx{i}", name=f"vpx{i}")
               for i in range(10)]
        for i in range(10):
            o4 = vpx[i][:].rearrange("p (b t c) -> p b t c", t=2, c=128)
            nc.gpsimd.memset(o4[:, :, :, 64:128], 1.0)

        # packed exp-score strips: es[(j, buf)] [128, 768]
        ESB = 4
        es = {(j, bf): sb.tile([128, ES_W], F16, tag=f"es{j}_{bf}",
                               name=f"es{j}_{bf}")
              for j in range(2) for bf in range(ESB)}

        # PE warm-up: dummy matmuls spanning the DMA prologue so the HAM
        # clock gate reaches 8/8 before the real matmuls arrive.
        warm = pp.tile([128, 128], F32, tag="gp", bufs=2, name="warm")
        for _ in range(44):
            nc.tensor.matmul(warm[:], ones_f[:], ones_f[:], start=True, stop=True)

        # ---- projections (emitted per head-pair, interleaved below) ----
        def copy_psum(dst, src):
            # projection copies on DVE: the ACT queue is the softmax critical
            # path (exp + ln); gpsimd cannot read PSUM on TRN2
            nc.vector.tensor_copy(dst, src)

        def proj_qk(pr, which):
            """q (which=0) or k (which=1) projection for head pair pr."""
            ft = pr if which == 0 else 4 + pr
            if which == 0:
                chunks = [(128, 512), (640, 512)]
                dest, doff = qT[pr], -128
            else:
                chunks = [(0, 512), (512, 512), (1024, 256)]
                dest, doff = kT[pr], 0
            for r0, rw in chunks:
                mm = pp.tile([128, C], F32, tag="gp", bufs=2, name="mmqk")
                for ct in range(4):
                    nc.tensor.matmul(
                        mm[:, 0:rw],
                        wq_sb[ct][:, 128 * ft:128 * (ft + 1)],
                        xT[ct][:, r0:r0 + rw],
                        start=(ct == 0), stop=(ct == 3))
                copy_psum(dest[:, r0 + doff:r0 + doff + rw], mm[:, 0:rw])

        def proj_v(rt):
            """v projection for key-row-tile rt -> vpx[rt] (all 4 pairs)."""
            mm = pp.tile([128, C], F32, tag="gp", bufs=2, name="mmv")
            for ct in range(4):
                nc.tensor.matmul(
                    mm[:],
                    xT[ct][:, 128 * rt:128 * (rt + 1)],
                    wq_sb[ct][:, 1024:1536],
                    start=(ct == 0), stop=(ct == 3))
            src = mm[:].rearrange("p (b t c) -> p b t c", t=2, c=HD)
            dst = vpx[rt][:].rearrange("p (b t c) -> p b t c", t=2, c=128)
            copy_psum(dst[:, :, :, 0:64], src[:, :, :, :])

        # ---- attention per (pr, qc) slot; everything double-buffered ----
        oall = [[None] * 4 for _ in range(NQC)]

        def attn_S(pr, qc, fillers, it):
            """S matmuls -> exp -> band-mask for one 256-query window."""
            esb = it % ESB
            for j in range(2):
                p0 = 64 * j
                e_t = es[(j, esb)]
                # both score banks in one 2-bank psum tile; ONE strided exp
                sp2 = pp.tile([128, 1024], F32, tag="sp", bufs=2, name="sp")
                for kt in range(NKT):
                    tc0, tc1 = _KT_TC[kt]
                    bk, c0 = _KT_BANK[kt]
                    kcol = QW * qc + 128 * kt
                    nc.tensor.matmul(
                        sp2[:, 512 * bk + c0:512 * bk + c0 + (tc1 - tc0)],
                        kT[pr][p0:p0 + 64, kcol:kcol + 128],
                        qT[pr][p0:p0 + 64, QW * qc + tc0:QW * qc + tc1],
                        start=True, stop=True)
                sp_v = sp2[:].rearrange("p (b c) -> p b c", c=512)[:, :, 0:384]
                es_v = e_t[:].rearrange("p (b c) -> p b c", c=384)
                nc.scalar.activation(es_v, sp_v, EXP)
                half = fillers[:1] if j == 0 else fillers[1:]
                for f in half:
                    f()
            # band-mask multiply: one head strip on the otherwise-idle
            # gpsimd, the other on DVE (f16 2x mode)
            m0 = ES_W * qc
            nc.gpsimd.tensor_mul(es[(0, esb)][:], es[(0, esb)][:],
                                 msk[:, m0:m0 + ES_W])
            nc.vector.tensor_mul(es[(1, esb)][:], es[(1, esb)][:],
                                 msk[:, m0:m0 + ES_W])

        def attn_AV(pr, qc, it):
            """AV matmuls + ln/exp normalize; lags attn_S by two slots."""
            esb = it % ESB
            # one bank holds [O|sums] for BOTH heads: j=0 -> cols [0:256),
            # j=1 -> [256:512); a single shared opener keeps the sim's
            # bank-granular pending-zero tracking uniform.
            otp = pp.tile([128, C], F32, tag="av", bufs=2, name="otp")
            nc.tensor.matmul(otp[:], zline[0:1, 0:128], zline[:],
                             start=True, stop=False)
            for j in range(2):
                e_t = es[(j, esb)]
                b0 = QW * j
                for kt in range(NKT):
                    tc0, tc1 = _KT_TC[kt]
                    nc.tensor.matmul(
                        otp[:, b0 + tc0:b0 + tc1],
                        vpx[2 * qc + kt][:, 256 * pr + 128 * j:
                                         256 * pr + 128 * j + 128],
                        e_t[:, _KT_OFF[kt]:_KT_OFF[kt] + (tc1 - tc0)],
                        start=False, stop=(j == 1 and kt == NKT - 1))
            oa = sb.tile([128, QW], F16, tag=f"oa{qc}_{pr}", name=f"oa{qc}_{pr}")
            rs = sb.tile([64, C], F32, tag="rs", bufs=2, name="rs")
            # 1/sums = exp(-ln(sums)): same ACT table as the score exps
            nc.scalar.activation(rs[:], otp[64:128, :],
                                 mybir.ActivationFunctionType.Ln)
            nc.scalar.activation(rs[:], rs[:], EXP, scale=-1.0)
            nc.vector.tensor_mul(oa[0:64, :], otp[0:64, 0:256], rs[:, 0:256])
            nc.vector.tensor_mul(oa[64:128, :], otp[0:64, 256:512],
                                 rs[:, 256:512])
            oall[qc][pr] = oa
            if dbg is not None and pr == 0 and qc == 0:
                ssd = sb.tile([64, C], F32, tag="ssd", name="ssd")
                nc.vector.tensor_copy(ssd[:], otp[64:128, :])
                nc.sync.dma_start(dbg["sums00"][:], ssd[:])
                nc.sync.dma_start(dbg["es00"][:], es[(0, esb)][:])
                nc.sync.dma_start(dbg["oa00"][:], oa[:])

        def outproj(rb):
            """Output projection + bias + store for row-block rb (128 q)."""
            qc = rb // 2
            c0 = 128 * (rb % 2)
            yp = pp.tile([128, C], F32, tag="gp", bufs=2, name="yp")
            for pr in range(4):
                nc.tensor.matmul(
                    yp[:],
                    oall[qc][pr][:, c0:c0 + 128],
                    wo_sb[pr][:],
                    start=(pr == 0), stop=False)
            nc.tensor.matmul(yp[:], ones_b[:], bo[:], start=False, stop=True)
            ys = sb.tile([128, C], F16, tag="ys", bufs=3, name="ys")
            copy_psum(ys[:], yp[:])
            r0 = 128 * rb
            eng = nc.sync if rb % 2 == 0 else nc.scalar
            eng.dma_start(y[r0:r0 + 128, :], ys[:])

        # ---- emission schedule ----
        # 16 slots (pr-major over 4 windows); AV lags S by one slot.
        # Projections for pair pr+1 ride as fillers inside pr's slots; the
        # last slots carry the early output-projection row-blocks.
        proj_qk(0, 0)
        proj_qk(0, 1)
        for rt in range(4):
            proj_v(rt)
        fill = {
            0: [lambda: proj_v(4), lambda: proj_v(5)],
            1: [lambda: proj_v(6), lambda: proj_qk(1, 0)],
            2: [lambda: proj_v(7), lambda: proj_qk(1, 1)],
            3: [lambda: proj_v(8), lambda: proj_v(9)],
            5: [lambda: proj_qk(2, 0)],
            6: [lambda: proj_qk(2, 1)],
            8: [lambda: proj_qk(3, 0)],
            9: [lambda: proj_qk(3, 1)],
            # interleaved (3,qc) iterations below finish window qc early so
            # its output projections fill the late slots
            12: [lambda: outproj(0)],
            13: [lambda: outproj(1)],
            14: [lambda: outproj(2)],
            15: [lambda: outproj(3)],
        }
        iters = [(0, 0), (0, 1), (0, 2), (0, 3),
                 (1, 0), (1, 1), (1, 2), (1, 3),
                 (2, 0), (2, 1), (3, 0), (2, 2),
                 (3, 1), (2, 3), (3, 2), (3, 3)]
        for it, (pr, qc) in enumerate(iters):
            if it >= 2:
                attn_AV(*iters[it - 2], it - 2)
            attn_S(pr, qc, fill.get(it, []), it)
        attn_AV(*iters[14], 14)
        outproj(4)
        outproj(5)
        attn_AV(*iters[15], 15)
        outproj(6)
        outproj(7)


def build_nc():
    nc = bacc.Bacc("TRN2", target_bir_lowering=False, debug=False,
                   num_devices=NCORES)
    xh = nc.dram_tensor("xh", [C, ROWS], F16, kind="ExternalInput")
    wqkv = nc.dram_tensor("wqkv", [C, 3 * C], F16, kind="ExternalInput")
    wout = nc.dram_tensor("wout", [C, C], F16, kind="ExternalInput")
    bout = nc.dram_tensor("bout", [1, C], F16, kind="ExternalInput")
    masks = nc.dram_tensor("masks", [128, NQC * ES_W], F16, kind="ExternalInput")
    y = nc.dram_tensor("y", [CHUNK, C], F16, kind="ExternalOutput")
    dbg = None
    if DEBUG_DUMPS:
        dbg = {
            "sums00": nc.dram_tensor("sums00", [64, C], F32,
                                     kind="ExternalOutput")[:],
            "es00": nc.dram_tensor("es00", [128, ES_W], F16,
                                   kind="ExternalOutput")[:],
            "oa00": nc.dram_tensor("oa00", [128, QW], F16,
                                   kind="ExternalOutput")[:],
            "qT0": nc.dram_tensor("qT0", [128, CHUNK], F16,
                                  kind="ExternalOutput")[:],
            "kT0": nc.dram_tensor("kT0", [128, ROWS], F16,
                                  kind="ExternalOutput")[:],
            "vpx0": nc.dram_tensor("vpx0", [128, 1024], F16,
                                   kind="ExternalOutput")[:],
            "msk": nc.dram_tensor("mskout", [128, NQC * ES_W], F16,
                                  kind="ExternalOutput")[:],
        }
    with tile.TileContext(nc) as tc:
        build_attention_body(tc, y[:], xh[:], wqkv[:], wout[:], bout[:],
                             masks[:], dbg=dbg)
    nc.compile()
    return nc


def make_in_maps(x, W_qkv, W_out, b_out):
    """Shard the full inputs into 8 per-core input maps."""
    x = np.asarray(x, dtype=np.float32)
    wqkv = np.asarray(W_qkv, dtype=np.float32).copy()
    wqkv[:, :C] *= SCALE  # fold hd^-0.5 into the q projection
    wqkv = wqkv.astype(np.float16)
    wout = np.asarray(W_out, dtype=np.float32).astype(np.float16)
    bo = np.asarray(b_out, dtype=np.float32).astype(np.float16).reshape(1, C)
    in_maps = []
    for core in range(NCORES):
        b, ch = divmod(core, 4)
        qs = CHUNK * ch
        xhalo = np.zeros((ROWS, C), dtype=np.float16)
        g0, g1 = qs - WIN, qs + CHUNK + WIN
        s0, s1 = max(g0, 0), min(g1, T)
        xhalo[s0 - g0:s1 - g0, :] = x[b, s0:s1, :].astype(np.float16)
        xhalo = np.ascontiguousarray(xhalo.T)
        # band masks: [128 keys, 4*768] packed per (window, kt) true ranges,
        # with sequence-edge kills baked in.
        masks = np.zeros((128, NQC * ES_W), dtype=np.float16)
        p = np.arange(128)[:, None]
        for qc in range(NQC):
            for kt in range(NKT):
                tc0, tc1 = _KT_TC[kt]
                wt = tc1 - tc0
                cseg = np.arange(wt)[None, :]
                iq = QW * qc + tc0 + cseg          # local query index
                jk = QW * qc + 128 * kt - 128 + p  # local key index
                jg = qs + jk                       # global key index
                valid = (np.abs(iq - jk) <= WIN) & (jg >= 0) & (jg < T)
                masks[:, ES_W * qc + _KT_OFF[kt]:
                      ES_W * qc + _KT_OFF[kt] + wt] = valid
        in_maps.append(dict(xh=xhalo, wqkv=wqkv, wout=wout, bout=bo,
                            masks=masks))
    return in_maps


_CACHED_NC = None


def run_sharded(x, W_qkv, W_out, b_out, **run_kwargs):
    """Build (cached), run on 8 cores, gather. Returns (y_full, BassKernelResults)."""
    global _CACHED_NC
    if _CACHED_NC is None:
        _CACHED_NC = build_nc()
    in_maps = make_in_maps(x, W_qkv, W_out, b_out)
    res = run_bass_kernel_spmd(_CACHED_NC, in_maps, core_ids=list(range(NCORES)),
                               **run_kwargs)
    y_full = np.empty((B, T, C), dtype=np.float32)
    for core in range(NCORES):
        b, ch = divmod(core, 4)
        y_full[b, CHUNK * ch:CHUNK * (ch + 1), :] = \
            res.results[core]["y"].astype(np.float32)
    return y_full, res


def kernel(x, W_qkv, W_out, b_out):
    y, _ = run_sharded(x, W_qkv, W_out, b_out)
    return y

